# revision 32
# baseline (speedup 1.0000x reference)
"""Trainium2 Bass kernel: GQA causal self-attention block (B=1, T=2048, D=2048,
32 q-heads / 8 kv-heads, head_dim 64) with q/k/v/o projections.

Sharding: head-parallel (tensor parallel) across 8 NeuronCores.
Core c owns q-heads 4c..4c+3 and kv-head c:
  - computes Q^T/K^T (transposed, head-dim on partitions) and V (natural) for
    its heads from a host-pre-transposed x^T,
  - blockwise causal softmax(QK^T)V in a k-major layout (denominator obtained
    free via a ones-column appended to V),
  - a partial output projection out_c = ctx_c^T @ o_proj[rows_c, :].
The host sums the 8 partial outputs (the tensor-parallel reduction).

All data is bf16 (DRAM I/O, SBUF activations/weights); PSUM accumulation is
f32. bf16 matmuls run at full PE rate and halve DMA/SBUF traffic vs f32r.
ST matmuls on diagonal blocks are trimmed to the causal live columns.
PSUM layout: the attention 'st' pool sits at the bottom of the stack so the
attention ST/exp chain can overlap the projection phase, whose pools (and
later the ctx/o-proj pools) live in the top 4 banks.
"""

import os
import numpy as np

T = 2048
D = 2048
HQ, HKV = 32, 8
DH = 64
NCORES = 8
PAIRS = 2                 # 2 head-pairs per core (4 q heads)
NCH = D // 128            # 16 contraction chunks for projections
NTQ = 4                   # t-quarters in projection phase
TQW = T // NTQ            # 512
NQB = 4                   # q blocks of 512
QBW = 512
NKB = T // 128            # 16 k blocks of 128

_NC = None
LAST_RESULT = None


def build_nc():
    import concourse.tile as tile
    from concourse import bacc, mybir
    from concourse.masks import make_identity, make_upper_triangular

    f32 = mybir.dt.float32
    bf16 = mybir.dt.bfloat16
    Exp = mybir.ActivationFunctionType.Exp

    nc = bacc.Bacc("TRN2", target_bir_lowering=False, debug=False,
                   num_devices=NCORES)

    xt = nc.dram_tensor("xt", [D, T], bf16, kind="ExternalInput").ap()
    xtr = xt.rearrange("(g p) t -> p g t", p=128)   # [128, NCH, T]
    # weights host-permuted to partition-major so DMA lines are long
    qpt = nc.dram_tensor("qpt", [128, NCH, 4 * DH], bf16,
                         kind="ExternalInput").ap()
    kvpt = nc.dram_tensor("kvpt", [128, NCH, 2 * DH], bf16,
                          kind="ExternalInput").ap()
    opj = nc.dram_tensor("opj", [4 * DH, D], bf16, kind="ExternalInput").ap()
    out = nc.dram_tensor("out", [T, D], bf16, kind="ExternalOutput").ap()

    from contextlib import ExitStack
    with tile.TileContext(nc) as tc, ExitStack() as ctx:
        consts = ctx.enter_context(tc.tile_pool(name="consts", bufs=1))
        wpool = ctx.enter_context(tc.tile_pool(name="weights", bufs=1))
        qtp = ctx.enter_context(tc.tile_pool(name="qt", bufs=1))
        ktp = ctx.enter_context(tc.tile_pool(name="kt", bufs=1))
        vpool = ctx.enter_context(tc.tile_pool(name="v", bufs=1))
        xpool = ctx.enter_context(tc.tile_pool(name="xchunk", bufs=4))
        epool = ctx.enter_context(tc.tile_pool(name="exps", bufs=18))
        cpool = ctx.enter_context(tc.tile_pool(name="ctxsb", bufs=1))
        spool = ctx.enter_context(tc.tile_pool(name="stage", bufs=2))
        opool = ctx.enter_context(tc.tile_pool(name="outsb", bufs=3))
        rpool = ctx.enter_context(tc.tile_pool(name="recip", bufs=2))
        # attention ST psum at stack bottom (banks 0-3) so phase B's ST/exp
        # can overlap phase A (whose psum lives in banks 4-7)
        stp = ctx.enter_context(tc.tile_pool(name="st_ps", bufs=2,
                                             space="PSUM"))

        # weights tiles declared first so their first group DMAs can issue
        # before the constant build (PE's first matmul needs them)
        qpt_sb = wpool.tile([128, NCH, 4 * DH], bf16, tag="qpt")
        kvw_sb = wpool.tile([128, NCH, 2 * DH], bf16, tag="kvw")
        xc4_first = xpool.tile([128, 4, TQW], bf16, tag="xc", name="xc_first")
        nc.sync.dma_start(out=xc4_first[:, 0:1], in_=xtr[:, 0:1, 0:TQW])
        nc.sync.dma_start(out=qpt_sb[:, 0:2], in_=qpt[:, 0:2])
        nc.sync.dma_start(out=kvw_sb[:, 0:2], in_=kvpt[:, 0:2])
        nc.sync.dma_start(out=xc4_first[:, 1:4], in_=xtr[:, 1:4, 0:TQW])
        nc.sync.dma_start(out=qpt_sb[:, 2:8], in_=qpt[:, 2:8])
        nc.sync.dma_start(out=kvw_sb[:, 2:8], in_=kvpt[:, 2:8])

        # constants (built in f32 -- memset/affine_select write f32 -- then
        # converted to bf16 via tensor_copy)
        identf = consts.tile([128, 128], f32, tag="identf")
        make_identity(nc, identf)
        ident = consts.tile([128, 128], bf16, tag="ident")
        nc.vector.tensor_copy(out=ident, in_=identf)
        # mask[i, j] = 1.0 if i <= j else 0  (keep k_row <= q_col)
        maskf = consts.tile([128, 128], f32, tag="maskf")
        make_upper_triangular(nc, maskf, val=1.0, diag=True)
        mask = consts.tile([128, 128], bf16, tag="mask")
        nc.vector.tensor_copy(out=mask, in_=maskf)
        onesc = consts.tile([128, NKB], bf16, tag="onesc")
        nc.vector.memset(onesc, 1.0)
        onesr = consts.tile([65, 64], bf16, tag="onesr")
        nc.vector.memset(onesr, 1.0)

        # remaining weight group DMAs are interleaved into the phase-A loop
        # below, so the first matmul is not queued behind the whole load
        opj_r = opj.rearrange("(p r) j -> r p j", p=2)
        opj_sb = wpool.tile([128, 2, D], bf16, tag="opj")

        # activation storage
        # qt_sb[p]: rows 0-63 = head 2p (Q^T), rows 64-127 = head 2p+1
        qt_sb = [qtp.tile([128, T], bf16, tag=f"qt{p}", name=f"qt{p}")
                 for p in range(PAIRS)]
        # kv_sb: rows 0-63 = V^T, rows 64-127 = K^T  (kvpt = [v | k])
        kv_sb = ktp.tile([128, T], bf16, tag="kv")
        # K^T copy on partitions 0-63 (for the tile_position (0,0) ST matmul)
        kt_a = ktp.tile([64, T], bf16, tag="kta")
        # V natural [k, dh] per k-block, with a ones column at dh (denominator)
        v_sb = vpool.tile([128, NKB, DH + 1], bf16, tag="vsb")
        nc.vector.tensor_copy(out=v_sb[:, :, DH], in_=onesc)
        # per-pair stacked normalized ctx^T: rows 0-63 head 2p, 64-127 head 2p+1
        ctx_sb = [cpool.tile([128, T], bf16, tag=f"ctx{p}", name=f"ctxsb{p}")
                  for p in range(PAIRS)]

        # ---------------- helpers for interleaved emission ----------------
        pending_ex = {}

        def emit_stexp(qb, p, kb):
            q0 = QBW * qb
            kb_off = max(0, 128 * kb - q0)   # first causal-live q col
            st = stp.tile([128, 1024], f32, tag="st", name="st")
            nc.tensor.matmul(
                st[:, kb_off:512],
                lhsT=kt_a[:, 128 * kb:128 * kb + 128],
                rhs=qt_sb[p][0:64, q0 + kb_off:q0 + QBW],
                start=True, stop=True, tile_position=(0, 0))
            nc.tensor.matmul(
                st[:, 512 + kb_off:1024],
                lhsT=kv_sb[64:128, 128 * kb:128 * kb + 128],
                rhs=qt_sb[p][64:128, q0 + kb_off:q0 + QBW],
                start=True, stop=True, tile_position=(64, 0))
            ex = epool.tile([128, 1024], bf16, tag="ex", name="ex")
            if kb_off == 0:
                nc.scalar.activation(out=ex, in_=st, func=Exp)
            else:
                # one 3D-AP exp covering both heads' live columns; columns
                # below kb_off are never read by the ctx matmul
                st3 = st.rearrange("p (h q) -> p h q", h=2)
                ex3 = ex.rearrange("p (h q) -> p h q", h=2)
                nc.scalar.activation(
                    out=ex3[:, :, kb_off:512],
                    in_=st3[:, :, kb_off:512], func=Exp)
            if 128 * kb >= q0:  # diagonal block: causal mask (both heads
                # in one 3D-AP multiply; mask broadcast along the head dim)
                ex3m = ex.rearrange("p (h q) -> p h q", h=2)
                nc.vector.tensor_mul(
                    ex3m[:, :, kb_off:kb_off + 128],
                    ex3m[:, :, kb_off:kb_off + 128],
                    mask.rearrange("p (h w) -> p h w", h=1).to_broadcast(
                        [128, 2, 128]))
            return ex

        # ---------------- Phase A: projections (t-quarters) ----------------
        with tc.tile_pool(name="pa_ps", bufs=1, space="PSUM") as pa:
            for tq in range(NTQ):
                t0 = TQW * tq
                qt_ps = [pa.tile([128, TQW], f32, tag=f"qtps{m}",
                                 name=f"qtps{m}") for m in range(2)]
                kv_ps = pa.tile([128, TQW], f32, tag="kvps")
                for g in range(NCH // 4):
                    if tq == 0 and g == 2:
                        nc.sync.dma_start(out=qpt_sb[:, 8:16],
                                          in_=qpt[:, 8:16])
                        nc.sync.dma_start(out=kvw_sb[:, 8:16],
                                          in_=kvpt[:, 8:16])
                    if tq == 0 and g == 0:
                        xc4 = xc4_first
                    else:
                        xc4 = xpool.tile([128, 4, TQW], bf16, tag="xc")
                        nc.sync.dma_start(
                            out=xc4,
                            in_=xtr[:, 4 * g:4 * g + 4, t0:t0 + TQW])
                    for i in range(4):
                        ci = 4 * g + i
                        for m in range(2):
                            nc.tensor.matmul(
                                qt_ps[m],
                                lhsT=qpt_sb[:, ci, 128 * m:128 * m + 128],
                                rhs=xc4[:, i, :],
                                start=(ci == 0), stop=(ci == NCH - 1))
                        nc.tensor.matmul(
                            kv_ps, lhsT=kvw_sb[:, ci, :], rhs=xc4[:, i, :],
                            start=(ci == 0), stop=(ci == NCH - 1))
                nc.vector.tensor_copy(out=qt_sb[0][:, t0:t0 + TQW],
                                      in_=qt_ps[0])
                nc.scalar.copy(out=qt_sb[1][:, t0:t0 + TQW], in_=qt_ps[1])
                nc.vector.tensor_copy(out=kv_sb[:, t0:t0 + TQW], in_=kv_ps)
                # K^T duplicate for this quarter (cross-partition SBUF DMA)
                nc.sync.dma_start(out=kt_a[:, t0:t0 + TQW],
                                  in_=kv_sb[64:128, t0:t0 + TQW])
                # pre-emit attention ST/exp units for this quarter; they
                # interleave with the V transposes below so the PE never
                # stalls on the single-buffer vtr psum -> v_sb copy chain
                units = {0: [(0, p, kb) for p in range(PAIRS)
                             for kb in range(4)],
                         1: [(1, 0, kb) for kb in range(4)],
                         2: [(1, 1, kb) for kb in range(4)],
                         3: [(2, 0, kb) for kb in range(4)]}[tq]
                units = list(units)
                # V natural via PE transpose of this quarter's V^T blocks
                for c in range(4 * tq, 4 * tq + 4):
                    tp = pa.tile([128, 64], bf16, tag="vtr", name="vtr")
                    nc.tensor.transpose(
                        tp, in_=kv_sb[0:64, 128 * c:128 * c + 128],
                        identity=ident[0:64, 0:64])
                    nc.vector.tensor_copy(out=v_sb[:, c, 0:DH], in_=tp)
                    if units:
                        u = units.pop(0)
                        pending_ex[u] = emit_stexp(*u)
                while units:
                    u = units.pop(0)
                    pending_ex[u] = emit_stexp(*u)

        # o_proj weights (first needed by phase C)
        nc.sync.dma_start(out=opj_sb, in_=opj_r)

        # ---------------- Phase B (attention) + C (o_proj) ----------------
        with tc.tile_pool(name="ctx_ps", bufs=2, space="PSUM") as cxp, \
             tc.tile_pool(name="oc_ps", bufs=2, space="PSUM") as ocp:
            # C-unit state: emit o_proj tiles of the previous qb in drips
            cstate = {"units": [], "osb": None, "tt": -1}

            def emit_cunit(drain=False):
                if not cstate["units"]:
                    return
                tt, jn = cstate["units"].pop(0)
                if cstate["tt"] != tt:
                    cstate["osb"] = opool.tile([128, D], bf16, tag="osb",
                                               name="osb")
                    cstate["tt"] = tt
                osb = cstate["osb"]
                oc = ocp.tile([128, 512], f32, tag="oc", name="oc")
                for p in range(PAIRS):
                    nc.tensor.matmul(
                        oc,
                        lhsT=ctx_sb[p][:, 128 * tt:128 * tt + 128],
                        rhs=opj_sb[:, p, 512 * jn:512 * jn + 512],
                        start=(p == 0), stop=(p == PAIRS - 1))
                # in the final drain there are no more exps: borrow ACT for
                # half the psum->sbuf copies, and split the row DMA in two
                # so the first half streams out during the second half
                if drain and jn % 2 == 1:
                    nc.scalar.copy(
                        out=osb[:, 512 * jn:512 * jn + 512], in_=oc)
                else:
                    nc.vector.tensor_copy(
                        out=osb[:, 512 * jn:512 * jn + 512], in_=oc)
                if drain:
                    if jn == 1:
                        nc.sync.dma_start(
                            out=out[128 * tt:128 * tt + 128, 0:1024],
                            in_=osb[:, 0:1024])
                    elif jn == 3:
                        nc.sync.dma_start(
                            out=out[128 * tt:128 * tt + 128, 1024:2048],
                            in_=osb[:, 1024:2048])
                elif jn == 3:
                    nc.sync.dma_start(
                        out=out[128 * tt:128 * tt + 128, :], in_=osb)

            for qb in range(NQB):
                q0 = QBW * qb
                nkb = 4 * qb + 4
                for p in range(PAIRS):
                    ctxh = [cxp.tile([DH + 1, 512], f32, tag="ctx",
                                     name=f"ctx{h}") for h in range(2)]

                    def emit_ctx(kb, ex):
                        # ctx^T (+ denominator row 64) accumulation; on
                        # diagonal blocks only cols >= kb_off can be nonzero
                        n0 = max(0, 128 * kb - q0)
                        for h in range(2):
                            o = 512 * h
                            nc.tensor.matmul(
                                ctxh[h][:, n0:512],
                                lhsT=v_sb[:, kb, :],
                                rhs=ex[:, o + n0:o + 512],
                                start=(kb == 0), stop=(kb == nkb - 1))

                    # one-unit software-pipeline skew: ST/exp of unit kb is
                    # emitted before ctx of unit kb-1, so the PE queue never
                    # head-of-line blocks on the ACT exp of the current unit
                    prev = None
                    for kb in range(nkb):
                        ex = pending_ex.pop((qb, p, kb), None)
                        if ex is None:
                            ex = emit_stexp(qb, p, kb)
                        if prev is not None:
                            emit_ctx(prev[0], prev[1])
                            # early in a qb's first pair the cunits still
                            # wait on the ctx_sb h1 DMA of the previous qb;
                            # dripping them then clogs the PE wait queue
                            if p == 1 or kb >= 3:
                                emit_cunit()
                        prev = (kb, ex)
                    emit_ctx(prev[0], prev[1])
                    emit_cunit()
                    for h in (1, 0):
                        # h=1 first: its result reaches ctx_sb via an SBUF
                        # DMA whose latency otherwise sits on the o_proj
                        # critical path at the last pair
                        # denominator row -> SBUF (bf16), replicate down 64
                        # partitions with a K=1 matmul, then reciprocal
                        densr = rpool.tile([65, 512], bf16, tag="densr")
                        nc.scalar.copy(
                            out=densr[64:65, :], in_=ctxh[h][64:65, :])
                        repl_ps = ocp.tile([64, 512], f32, tag="oc",
                                           name="replps")
                        nc.tensor.matmul(
                            repl_ps, lhsT=onesr[64:65, 0:64],
                            rhs=densr[64:65, :],
                            start=True, stop=True, tile_position=(64, 0))
                        repl = rpool.tile([64, 512], f32, tag="repl")
                        nc.vector.reciprocal(out=repl, in_=repl_ps)
                        if h == 0:
                            nc.vector.tensor_mul(
                                ctx_sb[p][0:64, q0:q0 + QBW],
                                ctxh[h][0:64, :], repl)
                        else:
                            stg = spool.tile([64, 512], bf16, tag="stg")
                            nc.vector.tensor_mul(
                                stg, ctxh[h][0:64, :], repl)
                            nc.sync.dma_start(
                                out=ctx_sb[p][64:128, q0:q0 + QBW], in_=stg)
                # queue this qb's o_proj tiles; leftovers of qb-1 carry
                # over and drip first during the coming qb
                cstate["units"] = (
                    cstate["units"]
                    + [(tt, jn) for tt in range(4 * qb, 4 * qb + 4)
                       for jn in range(4)])
            while cstate["units"]:
                emit_cunit(drain=True)

    nc.compile()
    return nc


def _get_nc():
    global _NC
    if _NC is None:
        _NC = build_nc()
    return _NC


def make_in_maps(x, q_proj, k_proj, v_proj, o_proj):
    import ml_dtypes
    bf = ml_dtypes.bfloat16
    x = np.asarray(x, np.float32).reshape(T, D)
    q_proj = np.asarray(q_proj, np.float32)
    k_proj = np.asarray(k_proj, np.float32)
    v_proj = np.asarray(v_proj, np.float32)
    o_proj = np.asarray(o_proj, np.float32)

    xt = np.ascontiguousarray(x.T).astype(bf)  # [D, T]
    scale = 1.0 / np.sqrt(np.float32(DH))

    def chunked(wT):
        # [D, n] -> [128, NCH, n] with row d = chunk ci*128 + partition p
        n = wT.shape[1]
        return np.ascontiguousarray(
            wT.reshape(NCH, 128, n).transpose(1, 0, 2)).astype(bf)

    maps = []
    for c in range(NCORES):
        qs = slice(4 * DH * c, 4 * DH * (c + 1))     # 256 q rows
        ks = slice(DH * c, DH * (c + 1))             # 64 kv rows
        m = {
            "xt": xt,
            "qpt": chunked(np.ascontiguousarray((q_proj[qs, :] * scale).T)),
            # [v | k]: V^T lands on partitions 0-63, K^T on 64-127
            "kvpt": chunked(np.ascontiguousarray(
                np.concatenate([v_proj[ks, :], k_proj[ks, :]], axis=0).T)),
            "opj": np.ascontiguousarray(o_proj[qs, :]).astype(bf),
        }
        maps.append(m)
    return maps


def kernel(**inputs):
    global LAST_RESULT
    from concourse.bass_utils import run_bass_kernel_spmd
    nc = _get_nc()
    maps = make_in_maps(inputs["x"], inputs["q_proj"], inputs["k_proj"],
                        inputs["v_proj"], inputs["o_proj"])
    res = run_bass_kernel_spmd(
        nc, maps, list(range(NCORES)),
        trace=bool(int(os.environ.get("BASS_KERNEL_TRACE", "0"))))
    LAST_RESULT = res
    acc = np.zeros((T, D), np.float32)
    for c in range(NCORES):
        acc += res.results[c]["out"].astype(np.float32)
    return acc.reshape(1, T, D)
